# revision 9
# baseline (speedup 1.0000x reference)
"""Trainium2 Bass kernel for nn_AttentionBlock (linear attention block).

Data-parallel over batch: core b computes batch b end-to-end (no collectives).

Math (per batch, heads h=8, dh=64, T=4096, F=256):
  qkv = w_qkv^T @ x                         (channel layout interleaved d*24+h*3+n)
  q,k = elu(.)+1 ; v raw
  cntxt[h] = k_h @ v_h^T  (contract T)      -> [64,64]
  norm[h,d] = sum_t q_h[d,t]*k_h[d,t]
  val[h]  = cntxt[h]^T @ q_h / (8*norm)     (per-row e scaling)
  out = w_out^T @ val_flat
Kernel folds cntxt, the 1/(8*norm) scaling, and w_out into a small
per-head matrix W2[hd, o] = sum_e cntxt[d,e] * w_out[he,o] / (8*norm[he]),
so the big T-dim epilogue is a single matmul: out = W2^T @ q.

Phase 1 (32 t-tiles of 128, software-pipelined with 1-tile skew so the PE
  never waits on ACT/DVE):
  slot tt: PE runs qkv matmuls for tile tt (x-stationary, c-outer so the
  stationary repeats), then norm + cntxt matmuls for tile tt-1 (whose
  elu outputs were produced by ACT/DVE during the previous slot).
  elu via one fused ACT Exp over [128,1024] (q|k) + one fused DVE
  ELU1SEL; v evicted on GpSimd (Pool).
Phase 2: norm recip (DMA round-trip transpose); scaled block-diag cntxtT;
  PE-transpose of all qT chunks -> q[hd,t] (replaces the DMA transposes
  that serialized the baseline); W2 matmuls interleaved mid-transposes.
Phase 3: out = W2^T @ q with w2-stationary reuse (16 LDWEIGHTS), PSUM
  eviction spread across DVE/ACT/Pool, DMA out.
"""

import sys, types

if "/opt/trn_rl_repo" not in sys.path:
    sys.path.insert(0, "/opt/trn_rl_repo")

import numpy as np
import ml_dtypes

# ---------------------------------------------------------------------------
# axon NTFF profile hook stub (lets run_bass_kernel_spmd(trace=True) work; the
# plain untraced path used for grading does not need it, but installing is
# harmless and lets any caller profile).
# ---------------------------------------------------------------------------
def _install_axon_hook_stub():
    try:
        import antenv
        if "antenv.axon_hooks" in sys.modules:
            return
        hooks = types.ModuleType("antenv.axon_hooks")
        hooks._hook = None
        def set_axon_ntff_profile_hook(h):
            hooks._hook = h
        def get_axon_ntff_profile_hook():
            return hooks._hook
        hooks.set_axon_ntff_profile_hook = set_axon_ntff_profile_hook
        hooks.get_axon_ntff_profile_hook = get_axon_ntff_profile_hook
        sys.modules["antenv.axon_hooks"] = hooks
        antenv.axon_hooks = hooks
        try:
            from trn_agent_boot.trn_boot import _ntff_profile_via_ctypes
            hooks._hook = _ntff_profile_via_ctypes("/opt/axon/libaxon_pjrt.so")
        except Exception:
            pass
    except Exception:
        pass

_install_axon_hook_stub()

import concourse.mybir as mybir
import concourse.tile as tile
from concourse import bacc, dve_ops
from concourse.bass_utils import run_bass_kernel_spmd
from concourse.dve_spec import Spec, Src0, Src1, Zero, One, select, lower
from concourse.dve_uop import DveOpSpec
from concourse.masks import make_identity

B, F, T = 8, 256, 4096
NH, DH = 8, 64
HID = NH * DH            # 512
NT = T // 128            # 32 t-tiles
NPAIR = 4                # head pairs (2 heads = 128 channels)
BF16 = mybir.dt.bfloat16
F32 = mybir.dt.float32
AF = mybir.ActivationFunctionType

# ---------------------------------------------------------------------------
# custom DVE op: out = x > 0 ? x+1 : e   (e = exp(x) supplied by ScalarE)
# ---------------------------------------------------------------------------
def _register_elu_select():
    for op in dve_ops.OPS:
        if op.name == "ELU1SEL":
            return op
    spec = Spec(
        body=select(Src0 > Zero, Src0 + One, Src1),
        reference=lambda in0, in1, s0, s1, imm2: np.where(
            in0 > 0, in0.astype(np.float32) + 1.0, in1
        ).astype(np.float32),
    )
    shas = {}
    for ver in ("v3", "v4"):
        uops = lower(spec, ver=ver)
        shas[ver] = DveOpSpec(name="ELU1SEL", opcode=0, uops=uops, rd1_en=True).sha(ver)
    op = dve_ops.DveOp("ELU1SEL", spec, subdim=False, uops_sha=shas)
    dve_ops.OPS.append(op)
    dve_ops.CUSTOM_DVE_SPECS[op.name] = spec
    dve_ops._SUB_OPCODE_FOR_NAME[op.name] = max(dve_ops._SUB_OPCODE_FOR_NAME.values()) + 1
    return op

ELU1SEL = _register_elu_select()


def _build_kernel():
    nc = bacc.Bacc("TRN2", target_bir_lowering=False, debug=False, num_devices=8)

    x_d = nc.dram_tensor("x", [2, 128, T], BF16, kind="ExternalInput")
    wq_d = nc.dram_tensor("wq", [2, 128, HID], BF16, kind="ExternalInput")
    wk_d = nc.dram_tensor("wk", [2, 128, HID], BF16, kind="ExternalInput")
    wv_d = nc.dram_tensor("wv", [2, 128, HID], BF16, kind="ExternalInput")
    wo_d = nc.dram_tensor("wo", [4, 128, F], BF16, kind="ExternalInput")
    out_d = nc.dram_tensor("out", [2, 128, T], F32, kind="ExternalOutput")
    nscratch = nc.dram_tensor("nscratch", [1, HID], F32)

    with tile.TileContext(nc) as tc:
        with (
            tc.tile_pool(name="const", bufs=1) as constp,
            tc.tile_pool(name="wts", bufs=1) as wts,
            tc.tile_pool(name="xin", bufs=1) as xin,
            tc.tile_pool(name="qkbuf", bufs=1) as qkbuf,
            tc.tile_pool(name="qbuf", bufs=1) as qbuf,
            tc.tile_pool(name="work", bufs=3) as work,
            tc.tile_pool(name="ostage", bufs=4) as ostage,
        ):
            ones_sb = constp.tile([128, 1], BF16)
            nc.vector.memset(ones_sb[:], 1.0)
            zeros_sb = constp.tile([128, 128], BF16)
            nc.vector.memset(zeros_sb[:], 0.0)
            ident_sb = constp.tile([128, 128], BF16)
            make_identity(nc, ident_sb[:])

            # weights + x (x interleaved tch-major so both c-chunks of the
            # first tiles arrive first)
            wq_sb = wts.tile([128, 2, HID], BF16)
            wk_sb = wts.tile([128, 2, HID], BF16)
            wv_sb = wts.tile([128, 2, HID], BF16)
            wo_sb = wts.tile([128, 4, F], BF16)
            for c in range(2):
                nc.sync.dma_start(wq_sb[:, c, :], wq_d.ap()[c])
                nc.sync.dma_start(wk_sb[:, c, :], wk_d.ap()[c])
                nc.sync.dma_start(wv_sb[:, c, :], wv_d.ap()[c])
            for c in range(4):
                nc.sync.dma_start(wo_sb[:, c, :], wo_d.ap()[c])
            x_sb = xin.tile([128, 2, T], BF16)
            for tch in range(8):
                tsl = slice(tch * 512, (tch + 1) * 512)
                for c in range(2):
                    nc.sync.dma_start(x_sb[:, c, tsl], x_d.ap()[c][:, tsl])

            # persistent activations
            qkT = qkbuf.tile([128, NT, 2 * HID], BF16)   # [:, tt, 0:512]=qT, [:, tt, 512:1024]=kT
            q_sb = qbuf.tile([128, 4, T], BF16)          # q[hd, t], hd = c*128+p

            with tc.tile_pool(name="psB", bufs=1, space="PSUM") as psB:
                ctx_ps = psB.tile([128, NPAIR * 128], F32)   # cntxtT pair blocks (1 bank)
                norm_ps = psB.tile([1, HID], F32)            # ones^T @ (qT*kT)   (1 bank)

                # ---------------- phase 1 (1-tile software-pipeline skew) ----
                with tc.tile_pool(name="psA", bufs=2, space="PSUM") as psA:
                    ps_of = {}
                    vt_of = {}
                    pt_of = {}

                    def emit_qkv(tt):
                        # q|k|v PSUM, flat [128, 1536] = 3 banks
                        ps = psA.tile([128, 3 * HID], F32, tag="ps")
                        ps_of[tt] = ps
                        for c in range(2):
                            xs = x_sb[:, c, tt * 128:(tt + 1) * 128]
                            for j in range(3):
                                w_sb = (wq_sb, wk_sb, wv_sb)[j]
                                nc.tensor.matmul(
                                    ps[:, j * HID:(j + 1) * HID], xs, w_sb[:, c, :],
                                    start=(c == 0), stop=(c == 1))

                        # elu(x)+1 on q,k fused: one Exp + one DVE select over
                        # [128, 1024]
                        e_qk = work.tile([128, 2 * HID], BF16, tag="eqk")
                        nc.scalar.activation(e_qk[:], ps[:, 0:2 * HID], AF.Exp)
                        nc.vector._custom_dve(
                            ELU1SEL, out=qkT[:, tt, :],
                            in0=ps[:, 0:2 * HID], in1=e_qk[:])

                        vt = work.tile([128, HID], BF16, tag="vt")
                        # GPSIMD cannot read PSUM; ACT has the most slack
                        nc.scalar.activation(vt[:], ps[:, 2 * HID:3 * HID], AF.Copy)
                        vt_of[tt] = vt

                        p_t = work.tile([128, HID], BF16, tag="pt")
                        # SBUF-only op -> GpSimd (Pool), which is idle
                        nc.gpsimd.tensor_mul(
                            p_t[:], qkT[:, tt, 0:HID], qkT[:, tt, HID:2 * HID])
                        pt_of[tt] = p_t

                    def emit_reduce(tt):
                        # norm + cntxt matmuls for tile tt (inputs were
                        # produced during the previous PE slot)
                        kt_t = qkT[:, tt, HID:2 * HID]
                        nc.tensor.matmul(norm_ps[:], ones_sb[:], pt_of[tt][:],
                                         start=(tt == 0), stop=(tt == NT - 1))
                        if tt == 0:
                            # start=True clears has_written for the WHOLE bank,
                            # so it must happen exactly once for the shared ctx
                            # bank: write zeros across all 4 pair slots, then
                            # only accumulate.
                            nc.tensor.matmul(ctx_ps[:], zeros_sb[:], kt_t,
                                             start=True, stop=False)
                        vt = vt_of[tt]
                        for pr in range(NPAIR):
                            sl = slice(pr * 128, (pr + 1) * 128)
                            nc.tensor.matmul(
                                ctx_ps[:, sl], vt[:, sl],
                                qkT[:, tt, HID + pr * 128:HID + (pr + 1) * 128],
                                start=False, stop=(tt == NT - 1))
                        del ps_of[tt], vt_of[tt], pt_of[tt]

                    for tt in range(NT + 1):
                        if tt < NT:
                            emit_qkv(tt)
                        if tt >= 1:
                            emit_reduce(tt - 1)

                # ---------------- phase 2 ----------------
                # norm -> rscale = 1/(8*norm) transposed to [128, 4]
                norm8 = constp.tile([1, HID], F32)
                nc.scalar.activation(norm8[:], norm_ps[:], AF.Copy, scale=8.0)
                nc.sync.dma_start(nscratch.ap(), norm8[:])
                rsc_raw = constp.tile([128, 4], F32)
                nc.sync.dma_start(
                    rsc_raw[:], nscratch.ap().rearrange("a (j p) -> (a p) j", p=128)
                )
                rsc = constp.tile([128, 4], F32)
                nc.vector.reciprocal(rsc[:], rsc_raw[:])

                # scaled block-diagonal cntxtT (DVE)
                ctx_bd = wts.tile([128, NPAIR, 128], BF16)
                nc.vector.memset(ctx_bd[:], 0.0)
                for pr in range(NPAIR):
                    for hr in range(2):
                        rows = slice(hr * 64, (hr + 1) * 64)
                        cols = slice(pr * 128 + hr * 64, pr * 128 + (hr + 1) * 64)
                        nc.vector.tensor_scalar_mul(
                            ctx_bd[rows, pr, hr * 64:(hr + 1) * 64],
                            ctx_ps[rows, cols],
                            rsc[rows, pr:pr + 1],
                        )

                w2_sb = wts.tile([128, NPAIR, F], BF16)
                with (
                    tc.tile_pool(name="psT", bufs=2, space="PSUM") as psT,
                    tc.tile_pool(name="psW", bufs=2, space="PSUM") as psW,
                ):
                    # PE-transpose qT -> q[hd, t]. 8 transposes (2 t-tiles)
                    # share one PSUM bank, slot = c*2 + b, so a single
                    # [128,1024] copy evicts both tiles contiguously into
                    # q_sb's [4, T] layout. W2 matmuls slotted into the middle
                    # so their rsc/ctx_bd dependency wait hides behind
                    # transposes already queued on the PE.
                    def emit_transpose_pair(g):
                        pt = psT.tile([128, 8, 128], BF16, tag="tp")
                        for b in range(2):
                            tt = 2 * g + b
                            for c in range(4):
                                nc.tensor.transpose(
                                    pt[:, c * 2 + b, :],
                                    qkT[:, tt, c * 128:(c + 1) * 128],
                                    ident_sb[:])
                        dst = q_sb[:, :, 2 * g * 128:(2 * g + 2) * 128]
                        if g % 2 == 0:
                            nc.vector.tensor_copy(dst, pt[:])
                        else:
                            nc.scalar.activation(dst, pt[:], AF.Copy)

                    for g in range(8):
                        emit_transpose_pair(g)
                    for pr in range(NPAIR):
                        w2_ps = psW.tile([128, F], F32, tag="w2")
                        nc.tensor.matmul(w2_ps[:], ctx_bd[:, pr, :], wo_sb[:, pr, :],
                                         start=True, stop=True)
                        nc.vector.tensor_copy(w2_sb[:, pr, :], w2_ps[:])
                    for g in range(8, 16):
                        emit_transpose_pair(g)

            # ---------------- phase 3 ----------------
            # out = W2^T @ q; w2 chunks stay stationary across 4 t-chunks
            # (16 LDWEIGHTS total), PSUM evictions round-robin DVE/ACT/Pool.
            with tc.tile_pool(name="psO", bufs=2, space="PSUM") as psO:
                ev = 0
                for oc in range(2):
                    for g in range(2):
                        po = psO.tile([128, 4, 512], F32, tag="po")
                        for c in range(4):
                            for ti in range(4):
                                tc_i = g * 4 + ti
                                tsl = slice(tc_i * 512, (tc_i + 1) * 512)
                                nc.tensor.matmul(
                                    po[:, ti, :],
                                    w2_sb[:, c, oc * 128:(oc + 1) * 128],
                                    q_sb[:, c, tsl],
                                    start=(c == 0), stop=(c == 3),
                                )
                        for ti in range(4):
                            tc_i = g * 4 + ti
                            tsl = slice(tc_i * 512, (tc_i + 1) * 512)
                            ot = ostage.tile([128, 512], F32, tag="ot")
                            if ev % 2 == 1:
                                nc.scalar.activation(ot[:], po[:, ti, :], AF.Copy)
                            else:
                                nc.vector.tensor_copy(ot[:], po[:, ti, :])
                            ev += 1
                            nc.sync.dma_start(out_d.ap()[oc, :, tsl], ot[:])

    nc.compile()
    return nc


_NC = None

def _get_nc():
    global _NC
    if _NC is None:
        _NC = _build_kernel()
    return _NC


def _prep_weights(w_qkv, w_out):
    """Host-side: un-interleave qkv columns to [h,d]-major, cast bf16, chunk."""
    d = np.arange(DH)[:, None]          # 64
    h = np.arange(NH)[None, :]          # 8
    # channel index in w_qkv for (h, d, n): d*24 + h*3 + n ; we want [h*64+d]
    def cols(n):
        c = (d * (NH * 3) + h * 3 + n)  # [64, 8]
        return c.T.reshape(-1)          # h-major: [h*64+d]
    bf = ml_dtypes.bfloat16
    wq = np.ascontiguousarray(w_qkv[:, cols(0)]).astype(bf).reshape(2, 128, HID)
    wk = np.ascontiguousarray(w_qkv[:, cols(1)]).astype(bf).reshape(2, 128, HID)
    wv = np.ascontiguousarray(w_qkv[:, cols(2)]).astype(bf).reshape(2, 128, HID)
    wo = np.ascontiguousarray(w_out).astype(bf).reshape(4, 128, F)
    return wq, wk, wv, wo


def kernel(x, w_qkv, w_out):
    x = np.asarray(x, dtype=np.float32)
    w_qkv = np.asarray(w_qkv, dtype=np.float32)
    w_out = np.asarray(w_out, dtype=np.float32)
    nc = _get_nc()
    wq, wk, wv, wo = _prep_weights(w_qkv, w_out)
    bf = ml_dtypes.bfloat16
    in_maps = []
    for b in range(B):
        xb = x[b].astype(bf).reshape(2, 128, T)
        in_maps.append({"x": xb, "wq": wq, "wk": wk, "wv": wv, "wo": wo})
    res = run_bass_kernel_spmd(nc, in_maps, core_ids=list(range(B)))
    out = np.empty((B, F, T), dtype=np.float32)
    for b in range(B):
        out[b] = res.results[b]["out"].reshape(F, T)
    return out


def run_traced(x, w_qkv, w_out):
    """Like kernel() but traced; returns (out, BassKernelResults)."""
    import concourse.bass_utils as bu
    bu.upload_artifacts = lambda tmpdir: tmpdir
    x = np.asarray(x, dtype=np.float32)
    nc = _get_nc()
    wq, wk, wv, wo = _prep_weights(np.asarray(w_qkv, np.float32), np.asarray(w_out, np.float32))
    bf = ml_dtypes.bfloat16
    in_maps = []
    for b in range(B):
        xb = x[b].astype(bf).reshape(2, 128, T)
        in_maps.append({"x": xb, "wq": wq, "wk": wk, "wv": wv, "wo": wo})
    res = run_bass_kernel_spmd(nc, in_maps, core_ids=list(range(B)), trace=True)
    out = np.empty((B, F, T), dtype=np.float32)
    for b in range(B):
        out[b] = res.results[b]["out"].reshape(F, T)
    return out, res


# revision 10
# speedup vs baseline: 1.0498x; 1.0498x over previous
"""Trainium2 Bass kernel for nn_AttentionBlock (linear attention block).

Data-parallel over batch: core b computes batch b end-to-end (no collectives).

Math (per batch, heads h=8, dh=64, T=4096, F=256):
  qkv = w_qkv^T @ x                         (channel layout interleaved d*24+h*3+n)
  q,k = elu(.)+1 ; v raw
  cntxt[h] = k_h @ v_h^T  (contract T)      -> [64,64]
  norm[h,d] = sum_t q_h[d,t]*k_h[d,t]
  val[h]  = cntxt[h]^T @ q_h / (8*norm)     (per-row e scaling)
  out = w_out^T @ val_flat
Kernel folds cntxt, the 1/(8*norm) scaling, and w_out into a small
per-head matrix W2[hd, o] = sum_e cntxt[d,e] * w_out[he,o] / (8*norm[he]),
so the big T-dim epilogue is a single matmul: out = W2^T @ q.

Phase 1 (32 t-tiles of 128, software-pipelined with 1-tile skew so the PE
  never waits on ACT/DVE):
  slot tt: PE runs qkv matmuls for tile tt (x-stationary, c-outer so the
  stationary repeats), then norm + cntxt matmuls for tile tt-1 (whose
  elu outputs were produced by ACT/DVE during the previous slot).
  elu via one fused ACT Exp over [128,1024] (q|k) + one fused DVE
  ELU1SEL; v evicted on GpSimd (Pool).
Phase 2: norm recip (DMA round-trip transpose); scaled block-diag cntxtT;
  PE-transpose of all qT chunks -> q[hd,t] (replaces the DMA transposes
  that serialized the baseline); W2 matmuls interleaved mid-transposes.
Phase 3: out = W2^T @ q with w2-stationary reuse (16 LDWEIGHTS), PSUM
  eviction spread across DVE/ACT/Pool, DMA out.
"""

import sys, types

if "/opt/trn_rl_repo" not in sys.path:
    sys.path.insert(0, "/opt/trn_rl_repo")

import numpy as np
import ml_dtypes

# ---------------------------------------------------------------------------
# axon NTFF profile hook stub (lets run_bass_kernel_spmd(trace=True) work; the
# plain untraced path used for grading does not need it, but installing is
# harmless and lets any caller profile).
# ---------------------------------------------------------------------------
def _install_axon_hook_stub():
    try:
        import antenv
        if "antenv.axon_hooks" in sys.modules:
            return
        hooks = types.ModuleType("antenv.axon_hooks")
        hooks._hook = None
        def set_axon_ntff_profile_hook(h):
            hooks._hook = h
        def get_axon_ntff_profile_hook():
            return hooks._hook
        hooks.set_axon_ntff_profile_hook = set_axon_ntff_profile_hook
        hooks.get_axon_ntff_profile_hook = get_axon_ntff_profile_hook
        sys.modules["antenv.axon_hooks"] = hooks
        antenv.axon_hooks = hooks
        try:
            from trn_agent_boot.trn_boot import _ntff_profile_via_ctypes
            hooks._hook = _ntff_profile_via_ctypes("/opt/axon/libaxon_pjrt.so")
        except Exception:
            pass
    except Exception:
        pass

_install_axon_hook_stub()

import concourse.mybir as mybir
import concourse.tile as tile
from concourse import bacc, dve_ops
from concourse.bass_utils import run_bass_kernel_spmd
from concourse.dve_spec import Spec, Src0, Src1, Zero, One, select, lower
from concourse.dve_uop import DveOpSpec
from concourse.masks import make_identity

B, F, T = 8, 256, 4096
NH, DH = 8, 64
HID = NH * DH            # 512
NT = T // 128            # 32 t-tiles
NPAIR = 4                # head pairs (2 heads = 128 channels)
BF16 = mybir.dt.bfloat16
F32 = mybir.dt.float32
AF = mybir.ActivationFunctionType

# ---------------------------------------------------------------------------
# custom DVE op: out = x > 0 ? x+1 : e   (e = exp(x) supplied by ScalarE)
# ---------------------------------------------------------------------------
def _register_elu_select():
    for op in dve_ops.OPS:
        if op.name == "ELU1SEL":
            return op
    spec = Spec(
        body=select(Src0 > Zero, Src0 + One, Src1),
        reference=lambda in0, in1, s0, s1, imm2: np.where(
            in0 > 0, in0.astype(np.float32) + 1.0, in1
        ).astype(np.float32),
    )
    shas = {}
    for ver in ("v3", "v4"):
        uops = lower(spec, ver=ver)
        shas[ver] = DveOpSpec(name="ELU1SEL", opcode=0, uops=uops, rd1_en=True).sha(ver)
    op = dve_ops.DveOp("ELU1SEL", spec, subdim=False, uops_sha=shas)
    dve_ops.OPS.append(op)
    dve_ops.CUSTOM_DVE_SPECS[op.name] = spec
    dve_ops._SUB_OPCODE_FOR_NAME[op.name] = max(dve_ops._SUB_OPCODE_FOR_NAME.values()) + 1
    return op

ELU1SEL = _register_elu_select()


def _build_kernel():
    nc = bacc.Bacc("TRN2", target_bir_lowering=False, debug=False, num_devices=8)

    x_d = nc.dram_tensor("x", [2, 128, T], BF16, kind="ExternalInput")
    wq_d = nc.dram_tensor("wq", [2, 128, HID], BF16, kind="ExternalInput")
    wk_d = nc.dram_tensor("wk", [2, 128, HID], BF16, kind="ExternalInput")
    wv_d = nc.dram_tensor("wv", [2, 128, HID], BF16, kind="ExternalInput")
    wo_d = nc.dram_tensor("wo", [4, 128, F], BF16, kind="ExternalInput")
    out_d = nc.dram_tensor("out", [2, 128, T], F32, kind="ExternalOutput")
    nscratch = nc.dram_tensor("nscratch", [1, HID], F32)

    with tile.TileContext(nc) as tc:
        with (
            tc.tile_pool(name="const", bufs=1) as constp,
            tc.tile_pool(name="wts", bufs=1) as wts,
            tc.tile_pool(name="xin", bufs=1) as xin,
            tc.tile_pool(name="qkbuf", bufs=1) as qkbuf,
            tc.tile_pool(name="qbuf", bufs=1) as qbuf,
            tc.tile_pool(name="work", bufs=3) as work,
            tc.tile_pool(name="ostage", bufs=4) as ostage,
        ):
            ones_sb = constp.tile([128, 1], BF16)
            nc.vector.memset(ones_sb[:], 1.0)
            zeros_sb = constp.tile([128, 128], BF16)
            nc.vector.memset(zeros_sb[:], 0.0)
            ident_sb = constp.tile([128, 128], BF16)
            make_identity(nc, ident_sb[:])

            # weights + x (x interleaved tch-major so both c-chunks of the
            # first tiles arrive first)
            wq_sb = wts.tile([128, 2, HID], BF16)
            wk_sb = wts.tile([128, 2, HID], BF16)
            wv_sb = wts.tile([128, 2, HID], BF16)
            wo_sb = wts.tile([128, 4, F], BF16)
            for c in range(2):
                nc.sync.dma_start(wq_sb[:, c, :], wq_d.ap()[c])
                nc.sync.dma_start(wk_sb[:, c, :], wk_d.ap()[c])
                nc.sync.dma_start(wv_sb[:, c, :], wv_d.ap()[c])
            for c in range(4):
                nc.sync.dma_start(wo_sb[:, c, :], wo_d.ap()[c])
            x_sb = xin.tile([128, 2, T], BF16)
            for tch in range(8):
                tsl = slice(tch * 512, (tch + 1) * 512)
                for c in range(2):
                    nc.sync.dma_start(x_sb[:, c, tsl], x_d.ap()[c][:, tsl])

            # persistent activations
            qkT = qkbuf.tile([128, NT, 2 * HID], BF16)   # [:, tt, 0:512]=qT, [:, tt, 512:1024]=kT
            q_sb = qbuf.tile([128, 4, T], BF16)          # q[hd, t], hd = c*128+p

            with tc.tile_pool(name="psB", bufs=1, space="PSUM") as psB:
                ctx_ps = psB.tile([128, NPAIR * 128], F32)   # cntxtT pair blocks (1 bank)
                norm_ps = psB.tile([1, HID], F32)            # ones^T @ (qT*kT)   (1 bank)

                # ---------------- phase 1 (1-tile software-pipeline skew) ----
                with tc.tile_pool(name="psA", bufs=2, space="PSUM") as psA:
                    ps_of = {}
                    vt_of = {}
                    pt_of = {}

                    def emit_qkv(tt):
                        # q|k|v PSUM, flat [128, 1536] = 3 banks
                        ps = psA.tile([128, 3 * HID], F32, tag="ps")
                        ps_of[tt] = ps
                        for c in range(2):
                            xs = x_sb[:, c, tt * 128:(tt + 1) * 128]
                            for j in range(3):
                                w_sb = (wq_sb, wk_sb, wv_sb)[j]
                                nc.tensor.matmul(
                                    ps[:, j * HID:(j + 1) * HID], xs, w_sb[:, c, :],
                                    start=(c == 0), stop=(c == 1))

                        # elu(x)+1 on q,k fused: one Exp + one DVE select over
                        # [128, 1024]
                        e_qk = work.tile([128, 2 * HID], BF16, tag="eqk")
                        nc.scalar.activation(e_qk[:], ps[:, 0:2 * HID], AF.Exp)
                        nc.vector._custom_dve(
                            ELU1SEL, out=qkT[:, tt, :],
                            in0=ps[:, 0:2 * HID], in1=e_qk[:])

                        vt = work.tile([128, HID], BF16, tag="vt")
                        # GPSIMD cannot read PSUM; ACT has the most slack
                        nc.scalar.activation(vt[:], ps[:, 2 * HID:3 * HID], AF.Copy)
                        vt_of[tt] = vt

                        p_t = work.tile([128, HID], BF16, tag="pt")
                        nc.vector.tensor_mul(
                            p_t[:], qkT[:, tt, 0:HID], qkT[:, tt, HID:2 * HID])
                        pt_of[tt] = p_t

                    def emit_reduce(tt):
                        # norm + cntxt matmuls for tile tt (inputs were
                        # produced during the previous PE slot)
                        kt_t = qkT[:, tt, HID:2 * HID]
                        nc.tensor.matmul(norm_ps[:], ones_sb[:], pt_of[tt][:],
                                         start=(tt == 0), stop=(tt == NT - 1))
                        if tt == 0:
                            # start=True clears has_written for the WHOLE bank,
                            # so it must happen exactly once for the shared ctx
                            # bank: write zeros across all 4 pair slots, then
                            # only accumulate.
                            nc.tensor.matmul(ctx_ps[:], zeros_sb[:], kt_t,
                                             start=True, stop=False)
                        vt = vt_of[tt]
                        for pr in range(NPAIR):
                            sl = slice(pr * 128, (pr + 1) * 128)
                            nc.tensor.matmul(
                                ctx_ps[:, sl], vt[:, sl],
                                qkT[:, tt, HID + pr * 128:HID + (pr + 1) * 128],
                                start=False, stop=(tt == NT - 1))
                        del ps_of[tt], vt_of[tt], pt_of[tt]

                    for tt in range(NT + 1):
                        if tt < NT:
                            emit_qkv(tt)
                        if tt >= 1:
                            emit_reduce(tt - 1)

                # ---------------- phase 2 ----------------
                # norm -> rscale = 1/(8*norm) transposed to [128, 4]
                norm8 = constp.tile([1, HID], F32)
                nc.scalar.activation(norm8[:], norm_ps[:], AF.Copy, scale=8.0)
                nc.sync.dma_start(nscratch.ap(), norm8[:])
                rsc_raw = constp.tile([128, 4], F32)
                nc.sync.dma_start(
                    rsc_raw[:], nscratch.ap().rearrange("a (j p) -> (a p) j", p=128)
                )
                rsc = constp.tile([128, 4], F32)
                nc.vector.reciprocal(rsc[:], rsc_raw[:])

                # scaled block-diagonal cntxtT (DVE)
                ctx_bd = wts.tile([128, NPAIR, 128], BF16)
                nc.vector.memset(ctx_bd[:], 0.0)
                for pr in range(NPAIR):
                    for hr in range(2):
                        rows = slice(hr * 64, (hr + 1) * 64)
                        cols = slice(pr * 128 + hr * 64, pr * 128 + (hr + 1) * 64)
                        nc.vector.tensor_scalar_mul(
                            ctx_bd[rows, pr, hr * 64:(hr + 1) * 64],
                            ctx_ps[rows, cols],
                            rsc[rows, pr:pr + 1],
                        )

                w2_sb = wts.tile([128, NPAIR, F], BF16)
                with (
                    tc.tile_pool(name="psT", bufs=2, space="PSUM") as psT,
                    tc.tile_pool(name="psW", bufs=2, space="PSUM") as psW,
                ):
                    # PE-transpose qT -> q[hd, t]. 8 transposes (2 t-tiles)
                    # share one PSUM bank, slot = c*2 + b, so a single
                    # [128,1024] copy evicts both tiles contiguously into
                    # q_sb's [4, T] layout. W2 matmuls slotted into the middle
                    # so their rsc/ctx_bd dependency wait hides behind
                    # transposes already queued on the PE.
                    def emit_transpose_pair(g):
                        pt = psT.tile([128, 8, 128], BF16, tag="tp")
                        for b in range(2):
                            tt = 2 * g + b
                            for c in range(4):
                                nc.tensor.transpose(
                                    pt[:, c * 2 + b, :],
                                    qkT[:, tt, c * 128:(c + 1) * 128],
                                    ident_sb[:])
                        dst = q_sb[:, :, 2 * g * 128:(2 * g + 2) * 128]
                        if g % 2 == 0:
                            nc.vector.tensor_copy(dst, pt[:])
                        else:
                            nc.scalar.activation(dst, pt[:], AF.Copy)

                    for g in range(8):
                        emit_transpose_pair(g)
                    for pr in range(NPAIR):
                        w2_ps = psW.tile([128, F], F32, tag="w2")
                        nc.tensor.matmul(w2_ps[:], ctx_bd[:, pr, :], wo_sb[:, pr, :],
                                         start=True, stop=True)
                        nc.vector.tensor_copy(w2_sb[:, pr, :], w2_ps[:])
                    for g in range(8, 16):
                        emit_transpose_pair(g)

            # ---------------- phase 3 ----------------
            # out = W2^T @ q; w2 chunks stay stationary across 4 t-chunks
            # (16 LDWEIGHTS total), PSUM evictions round-robin DVE/ACT/Pool.
            with tc.tile_pool(name="psO", bufs=2, space="PSUM") as psO:
                ev = 0
                for oc in range(2):
                    for g in range(2):
                        po = psO.tile([128, 4, 512], F32, tag="po")
                        for c in range(4):
                            for ti in range(4):
                                tc_i = g * 4 + ti
                                tsl = slice(tc_i * 512, (tc_i + 1) * 512)
                                nc.tensor.matmul(
                                    po[:, ti, :],
                                    w2_sb[:, c, oc * 128:(oc + 1) * 128],
                                    q_sb[:, c, tsl],
                                    start=(c == 0), stop=(c == 3),
                                )
                        for ti in range(4):
                            tc_i = g * 4 + ti
                            tsl = slice(tc_i * 512, (tc_i + 1) * 512)
                            ot = ostage.tile([128, 512], F32, tag="ot")
                            if ev % 2 == 1:
                                nc.scalar.activation(ot[:], po[:, ti, :], AF.Copy)
                            else:
                                nc.vector.tensor_copy(ot[:], po[:, ti, :])
                            ev += 1
                            nc.sync.dma_start(out_d.ap()[oc, :, tsl], ot[:])

    nc.compile()
    return nc


_NC = None

def _get_nc():
    global _NC
    if _NC is None:
        _NC = _build_kernel()
    return _NC


def _prep_weights(w_qkv, w_out):
    """Host-side: un-interleave qkv columns to [h,d]-major, cast bf16, chunk."""
    d = np.arange(DH)[:, None]          # 64
    h = np.arange(NH)[None, :]          # 8
    # channel index in w_qkv for (h, d, n): d*24 + h*3 + n ; we want [h*64+d]
    def cols(n):
        c = (d * (NH * 3) + h * 3 + n)  # [64, 8]
        return c.T.reshape(-1)          # h-major: [h*64+d]
    bf = ml_dtypes.bfloat16
    wq = np.ascontiguousarray(w_qkv[:, cols(0)]).astype(bf).reshape(2, 128, HID)
    wk = np.ascontiguousarray(w_qkv[:, cols(1)]).astype(bf).reshape(2, 128, HID)
    wv = np.ascontiguousarray(w_qkv[:, cols(2)]).astype(bf).reshape(2, 128, HID)
    wo = np.ascontiguousarray(w_out).astype(bf).reshape(4, 128, F)
    return wq, wk, wv, wo


def kernel(x, w_qkv, w_out):
    x = np.asarray(x, dtype=np.float32)
    w_qkv = np.asarray(w_qkv, dtype=np.float32)
    w_out = np.asarray(w_out, dtype=np.float32)
    nc = _get_nc()
    wq, wk, wv, wo = _prep_weights(w_qkv, w_out)
    bf = ml_dtypes.bfloat16
    in_maps = []
    for b in range(B):
        xb = x[b].astype(bf).reshape(2, 128, T)
        in_maps.append({"x": xb, "wq": wq, "wk": wk, "wv": wv, "wo": wo})
    res = run_bass_kernel_spmd(nc, in_maps, core_ids=list(range(B)))
    out = np.empty((B, F, T), dtype=np.float32)
    for b in range(B):
        out[b] = res.results[b]["out"].reshape(F, T)
    return out


def run_traced(x, w_qkv, w_out):
    """Like kernel() but traced; returns (out, BassKernelResults)."""
    import concourse.bass_utils as bu
    bu.upload_artifacts = lambda tmpdir: tmpdir
    x = np.asarray(x, dtype=np.float32)
    nc = _get_nc()
    wq, wk, wv, wo = _prep_weights(np.asarray(w_qkv, np.float32), np.asarray(w_out, np.float32))
    bf = ml_dtypes.bfloat16
    in_maps = []
    for b in range(B):
        xb = x[b].astype(bf).reshape(2, 128, T)
        in_maps.append({"x": xb, "wq": wq, "wk": wk, "wv": wv, "wo": wo})
    res = run_bass_kernel_spmd(nc, in_maps, core_ids=list(range(B)), trace=True)
    out = np.empty((B, F, T), dtype=np.float32)
    for b in range(B):
        out[b] = res.results[b]["out"].reshape(F, T)
    return out, res


# revision 11
# speedup vs baseline: 1.1024x; 1.0501x over previous
"""Trainium2 Bass kernel for nn_AttentionBlock (linear attention block).

Data-parallel over batch: core b computes batch b end-to-end (no collectives).

Math (per batch, heads h=8, dh=64, T=4096, F=256):
  qkv = w_qkv^T @ x                         (channel layout interleaved d*24+h*3+n)
  q,k = elu(.)+1 ; v raw
  cntxt[h] = k_h @ v_h^T  (contract T)      -> [64,64]
  norm[h,d] = sum_t q_h[d,t]*k_h[d,t]
  val[h]  = cntxt[h]^T @ q_h / (8*norm)     (per-row e scaling)
  out = w_out^T @ val_flat
Kernel folds cntxt, the 1/(8*norm) scaling, and w_out into a small
per-head matrix W2[hd, o] = sum_e cntxt[d,e] * w_out[he,o] / (8*norm[he]),
so the big T-dim epilogue is a single matmul: out = W2^T @ q.

Phase 1 (32 t-tiles of 128, software-pipelined with 1-tile skew so the PE
  never waits on ACT/DVE):
  slot tt: PE runs qkv matmuls for tile tt (x-stationary, c-outer so the
  stationary repeats), then norm + cntxt matmuls for tile tt-1 (whose
  elu outputs were produced by ACT/DVE during the previous slot).
  elu via one fused ACT Exp over [128,1024] (q|k) + one fused DVE
  ELU1SEL; v evicted on GpSimd (Pool).
Phase 2: norm recip (DMA round-trip transpose); scaled block-diag cntxtT;
  PE-transpose of all qT chunks -> q[hd,t] (replaces the DMA transposes
  that serialized the baseline); W2 matmuls interleaved mid-transposes.
Phase 3: out = W2^T @ q with w2-stationary reuse (16 LDWEIGHTS), PSUM
  eviction spread across DVE/ACT/Pool, DMA out.
"""

import sys, types

if "/opt/trn_rl_repo" not in sys.path:
    sys.path.insert(0, "/opt/trn_rl_repo")

import numpy as np
import ml_dtypes

# ---------------------------------------------------------------------------
# axon NTFF profile hook stub (lets run_bass_kernel_spmd(trace=True) work; the
# plain untraced path used for grading does not need it, but installing is
# harmless and lets any caller profile).
# ---------------------------------------------------------------------------
def _install_axon_hook_stub():
    try:
        import antenv
        if "antenv.axon_hooks" in sys.modules:
            return
        hooks = types.ModuleType("antenv.axon_hooks")
        hooks._hook = None
        def set_axon_ntff_profile_hook(h):
            hooks._hook = h
        def get_axon_ntff_profile_hook():
            return hooks._hook
        hooks.set_axon_ntff_profile_hook = set_axon_ntff_profile_hook
        hooks.get_axon_ntff_profile_hook = get_axon_ntff_profile_hook
        sys.modules["antenv.axon_hooks"] = hooks
        antenv.axon_hooks = hooks
        try:
            from trn_agent_boot.trn_boot import _ntff_profile_via_ctypes
            hooks._hook = _ntff_profile_via_ctypes("/opt/axon/libaxon_pjrt.so")
        except Exception:
            pass
    except Exception:
        pass

_install_axon_hook_stub()

import concourse.mybir as mybir
import concourse.tile as tile
from concourse import bacc, dve_ops
from concourse.bass_utils import run_bass_kernel_spmd
from concourse.dve_spec import Spec, Src0, Src1, Zero, One, select, lower
from concourse.dve_uop import DveOpSpec
from concourse.masks import make_identity

B, F, T = 8, 256, 4096
NH, DH = 8, 64
HID = NH * DH            # 512
NT = T // 128            # 32 t-tiles
NPAIR = 4                # head pairs (2 heads = 128 channels)
BF16 = mybir.dt.bfloat16
F32 = mybir.dt.float32
AF = mybir.ActivationFunctionType

# ---------------------------------------------------------------------------
# custom DVE op: out = x > 0 ? x+1 : e   (e = exp(x) supplied by ScalarE)
# ---------------------------------------------------------------------------
def _register_elu_select():
    for op in dve_ops.OPS:
        if op.name == "ELU1SEL":
            return op
    spec = Spec(
        body=select(Src0 > Zero, Src0 + One, Src1),
        reference=lambda in0, in1, s0, s1, imm2: np.where(
            in0 > 0, in0.astype(np.float32) + 1.0, in1
        ).astype(np.float32),
    )
    shas = {}
    for ver in ("v3", "v4"):
        uops = lower(spec, ver=ver)
        shas[ver] = DveOpSpec(name="ELU1SEL", opcode=0, uops=uops, rd1_en=True).sha(ver)
    op = dve_ops.DveOp("ELU1SEL", spec, subdim=False, uops_sha=shas)
    dve_ops.OPS.append(op)
    dve_ops.CUSTOM_DVE_SPECS[op.name] = spec
    dve_ops._SUB_OPCODE_FOR_NAME[op.name] = max(dve_ops._SUB_OPCODE_FOR_NAME.values()) + 1
    return op

ELU1SEL = _register_elu_select()


def _build_kernel():
    nc = bacc.Bacc("TRN2", target_bir_lowering=False, debug=False, num_devices=8)

    x_d = nc.dram_tensor("x", [2, 128, T], BF16, kind="ExternalInput")
    wq_d = nc.dram_tensor("wq", [2, 128, HID], BF16, kind="ExternalInput")
    wk_d = nc.dram_tensor("wk", [2, 128, HID], BF16, kind="ExternalInput")
    wv_d = nc.dram_tensor("wv", [2, 128, HID], BF16, kind="ExternalInput")
    wo_d = nc.dram_tensor("wo", [4, 128, F], BF16, kind="ExternalInput")
    out_d = nc.dram_tensor("out", [2, 128, T], F32, kind="ExternalOutput")
    nscratch = nc.dram_tensor("nscratch", [1, HID], F32)

    with tile.TileContext(nc) as tc:
        with (
            tc.tile_pool(name="const", bufs=1) as constp,
            tc.tile_pool(name="wts", bufs=1) as wts,
            tc.tile_pool(name="xin", bufs=1) as xin,
            tc.tile_pool(name="qkbuf", bufs=1) as qkbuf,
            tc.tile_pool(name="qbuf", bufs=1) as qbuf,
            tc.tile_pool(name="work", bufs=3) as work,
            tc.tile_pool(name="ostage", bufs=4) as ostage,
        ):
            ones_sb = constp.tile([128, 1], BF16)
            nc.vector.memset(ones_sb[:], 1.0)
            zeros_sb = constp.tile([128, 128], BF16)
            nc.vector.memset(zeros_sb[:], 0.0)
            ident_sb = constp.tile([128, 128], BF16)
            make_identity(nc, ident_sb[:])

            # weights + x (x interleaved tch-major so both c-chunks of the
            # first tiles arrive first)
            wq_sb = wts.tile([128, 2, HID], BF16)
            wk_sb = wts.tile([128, 2, HID], BF16)
            wv_sb = wts.tile([128, 2, HID], BF16)
            wo_sb = wts.tile([128, 4, F], BF16)
            for c in range(2):
                nc.sync.dma_start(wq_sb[:, c, :], wq_d.ap()[c])
                nc.sync.dma_start(wk_sb[:, c, :], wk_d.ap()[c])
                nc.sync.dma_start(wv_sb[:, c, :], wv_d.ap()[c])
            for c in range(4):
                nc.sync.dma_start(wo_sb[:, c, :], wo_d.ap()[c])
            x_sb = xin.tile([128, 2, T], BF16)
            for tch in range(8):
                tsl = slice(tch * 512, (tch + 1) * 512)
                for c in range(2):
                    nc.sync.dma_start(x_sb[:, c, tsl], x_d.ap()[c][:, tsl])

            # persistent activations
            qkT = qkbuf.tile([128, NT, 2 * HID], BF16)   # [:, tt, 0:512]=qT, [:, tt, 512:1024]=kT
            q_sb = qbuf.tile([128, 4, T], BF16)          # q[hd, t], hd = c*128+p

            with tc.tile_pool(name="psB", bufs=1, space="PSUM") as psB:
                ctx_ps = psB.tile([128, NPAIR * 128], F32)   # cntxtT pair blocks (1 bank)
                norm_ps = psB.tile([1, HID], F32)            # ones^T @ (qT*kT)   (1 bank)

                # ---------------- phase 1 (1-tile software-pipeline skew) ----
                with tc.tile_pool(name="psA", bufs=2, space="PSUM") as psA:
                    ps_of = {}
                    vt_of = {}
                    pt_of = {}

                    def emit_qkv(tt):
                        # q|k|v PSUM, flat [128, 1536] = 3 banks
                        ps = psA.tile([128, 3 * HID], F32, tag="ps")
                        ps_of[tt] = ps
                        for c in range(2):
                            xs = x_sb[:, c, tt * 128:(tt + 1) * 128]
                            for j in range(3):
                                w_sb = (wq_sb, wk_sb, wv_sb)[j]
                                nc.tensor.matmul(
                                    ps[:, j * HID:(j + 1) * HID], xs, w_sb[:, c, :],
                                    start=(c == 0), stop=(c == 1))

                        # elu(x)+1 on q,k fused: one Exp + one DVE select over
                        # [128, 1024]
                        e_qk = work.tile([128, 2 * HID], BF16, tag="eqk")
                        nc.scalar.activation(e_qk[:], ps[:, 0:2 * HID], AF.Exp)
                        nc.vector._custom_dve(
                            ELU1SEL, out=qkT[:, tt, :],
                            in0=ps[:, 0:2 * HID], in1=e_qk[:])

                        vt = work.tile([128, HID], BF16, tag="vt")
                        # GPSIMD cannot read PSUM; ACT has the most slack
                        nc.scalar.activation(vt[:], ps[:, 2 * HID:3 * HID], AF.Copy)
                        vt_of[tt] = vt

                        p_t = work.tile([128, HID], BF16, tag="pt")
                        nc.vector.tensor_mul(
                            p_t[:], qkT[:, tt, 0:HID], qkT[:, tt, HID:2 * HID])
                        pt_of[tt] = p_t

                    def emit_reduce(tt):
                        # norm + cntxt matmuls for tile tt (inputs were
                        # produced during the previous PE slot)
                        kt_t = qkT[:, tt, HID:2 * HID]
                        nc.tensor.matmul(norm_ps[:], ones_sb[:], pt_of[tt][:],
                                         start=(tt == 0), stop=(tt == NT - 1))
                        if tt == 0:
                            # start=True clears has_written for the WHOLE bank,
                            # so it must happen exactly once for the shared ctx
                            # bank: write zeros across all 4 pair slots, then
                            # only accumulate.
                            nc.tensor.matmul(ctx_ps[:], zeros_sb[:], kt_t,
                                             start=True, stop=False)
                        vt = vt_of[tt]
                        for pr in range(NPAIR):
                            sl = slice(pr * 128, (pr + 1) * 128)
                            nc.tensor.matmul(
                                ctx_ps[:, sl], vt[:, sl],
                                qkT[:, tt, HID + pr * 128:HID + (pr + 1) * 128],
                                start=False, stop=(tt == NT - 1))
                        del ps_of[tt], vt_of[tt], pt_of[tt]

                    for tt in range(NT + 1):
                        if tt < NT:
                            emit_qkv(tt)
                        if tt >= 1:
                            emit_reduce(tt - 1)

                # ---------------- phase 2 ----------------
                # norm -> rscale = 1/(8*norm) transposed to [128, 4]
                norm8 = constp.tile([1, HID], F32)
                nc.scalar.activation(norm8[:], norm_ps[:], AF.Copy, scale=8.0)
                nc.sync.dma_start(nscratch.ap(), norm8[:])
                rsc_raw = constp.tile([128, 4], F32)
                nc.sync.dma_start(
                    rsc_raw[:], nscratch.ap().rearrange("a (j p) -> (a p) j", p=128)
                )
                rsc = constp.tile([128, 4], F32)
                nc.vector.reciprocal(rsc[:], rsc_raw[:])

                # scaled block-diagonal cntxtT (DVE)
                ctx_bd = wts.tile([128, NPAIR, 128], BF16)
                nc.vector.memset(ctx_bd[:], 0.0)
                for pr in range(NPAIR):
                    for hr in range(2):
                        rows = slice(hr * 64, (hr + 1) * 64)
                        cols = slice(pr * 128 + hr * 64, pr * 128 + (hr + 1) * 64)
                        nc.vector.tensor_scalar_mul(
                            ctx_bd[rows, pr, hr * 64:(hr + 1) * 64],
                            ctx_ps[rows, cols],
                            rsc[rows, pr:pr + 1],
                        )

                w2_sb = wts.tile([128, NPAIR, F], BF16)
                with (
                    tc.tile_pool(name="psT", bufs=4, space="PSUM") as psT,
                    tc.tile_pool(name="psW", bufs=2, space="PSUM") as psW,
                ):
                    # PE-transpose qT -> q[hd, t]; W2 matmuls slotted into the
                    # middle so their rsc/ctx_bd dependency wait hides behind
                    # transposes already queued on the PE.
                    def emit_transpose(tt):
                        pt = psT.tile([128, 4, 128], BF16, tag="tp")
                        for c in range(4):
                            nc.tensor.transpose(
                                pt[:, c, :],
                                qkT[:, tt, c * 128:(c + 1) * 128],
                                ident_sb[:])
                        dst = q_sb[:, :, tt * 128:(tt + 1) * 128]
                        if tt % 2 == 0:
                            nc.vector.tensor_copy(dst, pt[:])
                        else:
                            nc.scalar.activation(dst, pt[:], AF.Copy)

                    for tt in range(16):
                        emit_transpose(tt)
                    for pr in range(NPAIR):
                        w2_ps = psW.tile([128, F], F32, tag="w2")
                        nc.tensor.matmul(w2_ps[:], ctx_bd[:, pr, :], wo_sb[:, pr, :],
                                         start=True, stop=True)
                        nc.vector.tensor_copy(w2_sb[:, pr, :], w2_ps[:])
                    for tt in range(16, NT):
                        emit_transpose(tt)

            # ---------------- phase 3 ----------------
            # out = W2^T @ q; w2 chunks stay stationary across 4 t-chunks
            # (16 LDWEIGHTS total), PSUM evictions round-robin DVE/ACT/Pool.
            with tc.tile_pool(name="psO", bufs=2, space="PSUM") as psO:
                ev = 0
                for oc in range(2):
                    for g in range(2):
                        po = psO.tile([128, 4, 512], F32, tag="po")
                        for c in range(4):
                            for ti in range(4):
                                tc_i = g * 4 + ti
                                tsl = slice(tc_i * 512, (tc_i + 1) * 512)
                                nc.tensor.matmul(
                                    po[:, ti, :],
                                    w2_sb[:, c, oc * 128:(oc + 1) * 128],
                                    q_sb[:, c, tsl],
                                    start=(c == 0), stop=(c == 3),
                                )
                        for ti in range(4):
                            tc_i = g * 4 + ti
                            tsl = slice(tc_i * 512, (tc_i + 1) * 512)
                            ot = ostage.tile([128, 512], F32, tag="ot")
                            if ev % 2 == 1:
                                nc.scalar.activation(ot[:], po[:, ti, :], AF.Copy)
                            else:
                                nc.vector.tensor_copy(ot[:], po[:, ti, :])
                            ev += 1
                            nc.sync.dma_start(out_d.ap()[oc, :, tsl], ot[:])

    nc.compile()
    return nc


_NC = None

def _get_nc():
    global _NC
    if _NC is None:
        _NC = _build_kernel()
    return _NC


def _prep_weights(w_qkv, w_out):
    """Host-side: un-interleave qkv columns to [h,d]-major, cast bf16, chunk."""
    d = np.arange(DH)[:, None]          # 64
    h = np.arange(NH)[None, :]          # 8
    # channel index in w_qkv for (h, d, n): d*24 + h*3 + n ; we want [h*64+d]
    def cols(n):
        c = (d * (NH * 3) + h * 3 + n)  # [64, 8]
        return c.T.reshape(-1)          # h-major: [h*64+d]
    bf = ml_dtypes.bfloat16
    wq = np.ascontiguousarray(w_qkv[:, cols(0)]).astype(bf).reshape(2, 128, HID)
    wk = np.ascontiguousarray(w_qkv[:, cols(1)]).astype(bf).reshape(2, 128, HID)
    wv = np.ascontiguousarray(w_qkv[:, cols(2)]).astype(bf).reshape(2, 128, HID)
    wo = np.ascontiguousarray(w_out).astype(bf).reshape(4, 128, F)
    return wq, wk, wv, wo


def kernel(x, w_qkv, w_out):
    x = np.asarray(x, dtype=np.float32)
    w_qkv = np.asarray(w_qkv, dtype=np.float32)
    w_out = np.asarray(w_out, dtype=np.float32)
    nc = _get_nc()
    wq, wk, wv, wo = _prep_weights(w_qkv, w_out)
    bf = ml_dtypes.bfloat16
    in_maps = []
    for b in range(B):
        xb = x[b].astype(bf).reshape(2, 128, T)
        in_maps.append({"x": xb, "wq": wq, "wk": wk, "wv": wv, "wo": wo})
    res = run_bass_kernel_spmd(nc, in_maps, core_ids=list(range(B)))
    out = np.empty((B, F, T), dtype=np.float32)
    for b in range(B):
        out[b] = res.results[b]["out"].reshape(F, T)
    return out


def run_traced(x, w_qkv, w_out):
    """Like kernel() but traced; returns (out, BassKernelResults)."""
    import concourse.bass_utils as bu
    bu.upload_artifacts = lambda tmpdir: tmpdir
    x = np.asarray(x, dtype=np.float32)
    nc = _get_nc()
    wq, wk, wv, wo = _prep_weights(np.asarray(w_qkv, np.float32), np.asarray(w_out, np.float32))
    bf = ml_dtypes.bfloat16
    in_maps = []
    for b in range(B):
        xb = x[b].astype(bf).reshape(2, 128, T)
        in_maps.append({"x": xb, "wq": wq, "wk": wk, "wv": wv, "wo": wo})
    res = run_bass_kernel_spmd(nc, in_maps, core_ids=list(range(B)), trace=True)
    out = np.empty((B, F, T), dtype=np.float32)
    for b in range(B):
        out[b] = res.results[b]["out"].reshape(F, T)
    return out, res
